# revision 1
# baseline (speedup 1.0000x reference)
import numpy as np
import jax
import jax.numpy as jnp

# nn_CausalLinearAttention: query (8, 512, 64, 128) f32; W* (128,128); b* (128,)
# Data-parallel over batch B=8 -> one batch element per NeuronCore (8 cores).
# Per core: chunked causal linear attention (fast_transformers style),
# feature map phi(x) = elu(x)+1, eps = 1e-6.

HEADS = 8
HEAD_DIM = 16
EPS = 1e-6
L = 512
N = 64
F = 128
C = 128          # time chunk
NC = L // C      # 4 chunks


def _per_device(xb, Wq, bq, Wk, bk, Wv, bv):
    # xb: (L, N, F) one batch element
    x = jnp.swapaxes(xb, 0, 1)                    # (N, L, F)
    q = jax.nn.elu(x @ Wq + bq) + 1.0             # (N, L, 128)
    k = jax.nn.elu(x @ Wk + bk) + 1.0
    v = x @ Wv + bv
    H, E = HEADS, HEAD_DIM
    qc = q.reshape(N, NC, C, H, E)
    kc = k.reshape(N, NC, C, H, E)
    vc = v.reshape(N, NC, C, H, E)

    # intra-chunk (diagonal blocks), causal mask incl. diagonal
    A = jnp.einsum('ncthe,ncshe->nchts', qc, kc)          # (N,NC,H,C,C)
    mask = jnp.tril(jnp.ones((C, C), dtype=x.dtype))
    Am = A * mask
    intra = jnp.einsum('nchts,ncshf->ncthf', Am, vc)      # (N,NC,C,H,E)
    den_intra = jnp.sum(Am, axis=-1)                      # (N,NC,H,C)
    den_intra = jnp.moveaxis(den_intra, 2, 3)             # (N,NC,C,H)

    # inter-chunk via exclusive cumulative KV state
    kv = jnp.einsum('ncshe,ncshf->nchef', kc, vc)         # (N,NC,H,E,E)
    S = jnp.cumsum(kv, axis=1) - kv                       # exclusive prefix
    inter = jnp.einsum('ncthe,nchef->ncthf', qc, S)       # (N,NC,C,H,E)

    ks = jnp.sum(kc, axis=2)                              # (N,NC,H,E)
    Ks = jnp.cumsum(ks, axis=1) - ks                      # exclusive prefix
    den_inter = jnp.einsum('ncthe,nche->ncth', qc, Ks)    # (N,NC,C,H)

    den = den_intra + den_inter + EPS                     # (N,NC,C,H)
    out = (intra + inter) / den[..., None]                # (N,NC,C,H,E)
    out = out.reshape(N, L, H * E)
    return jnp.swapaxes(out, 0, 1)                        # (L, N, 128)


_pmapped = None


def _get_pmapped():
    global _pmapped
    if _pmapped is None:
        _pmapped = jax.pmap(
            _per_device,
            in_axes=(0, None, None, None, None, None, None),
            devices=jax.devices()[:8],
        )
    return _pmapped


def kernel(query, Wq, bq, Wk, bk, Wv, bv):
    fn = _get_pmapped()
    out = fn(
        jnp.asarray(query, jnp.float32),
        jnp.asarray(Wq, jnp.float32), jnp.asarray(bq, jnp.float32),
        jnp.asarray(Wk, jnp.float32), jnp.asarray(bk, jnp.float32),
        jnp.asarray(Wv, jnp.float32), jnp.asarray(bv, jnp.float32),
    )
    return np.asarray(out, dtype=np.float32)



# revision 15
# speedup vs baseline: 1.7308x; 1.7308x over previous
"""Causal linear attention (fast_transformers style) on 8 Trainium2 cores.

query (8, 512, 64, 128) f32. Data-parallel: one batch element per core.
Per (batch, node) sequence of L=512 tokens: project q/k/v with 128x128
weights, phi(x)=elu(x)+1, causal linear attention via chunked scan
(C=128 intra-chunk masked matmul + inter-chunk running KV state).

Wire strategy: the axon tunnel moves ~75 MB/s and is the whole cost, so
ship fp16 both directions (67MB up, 67MB down). All matmuls run fp16 with
fp32 PSUM accumulation; constants/zero-buffers live on device across calls.
"""

import numpy as np

HEADS = 8
E = 16
EPS = 1e-6
L = 512
NSEQ = 64
F = 128
CH = HEADS * E  # 128 output channels
C = 128         # time chunk
NC = L // C
W17 = 17 * HEADS  # 136: per-head [num(16) | den(1)] interleaved width


def build_nc(n_seq=NSEQ, debug=False):
    """Build the per-core Bass module. Parametrized n_seq for simulation."""
    from contextlib import ExitStack

    import concourse.bacc as bacc
    import concourse.mybir as mybir
    import concourse.tile as tile

    f16 = mybir.dt.float16
    f32 = mybir.dt.float32
    Relu = mybir.ActivationFunctionType.Relu
    Exp = mybir.ActivationFunctionType.Exp
    AluOp = mybir.AluOpType

    nc = bacc.Bacc(
        "TRN2",
        target_bir_lowering=False,
        debug=debug,
        enable_asserts=False,
        num_devices=8,
    )

    xq = nc.dram_tensor("xq", (L * n_seq, F), f16, kind="ExternalInput").ap()
    wq = nc.dram_tensor("wq", (F, CH), f16, kind="ExternalInput").ap()
    wk = nc.dram_tensor("wk", (F, CH), f16, kind="ExternalInput").ap()
    wv = nc.dram_tensor("wv", (F, CH), f16, kind="ExternalInput").ap()
    bq = nc.dram_tensor("bq", (CH,), f16, kind="ExternalInput").ap()
    bk = nc.dram_tensor("bk", (CH,), f16, kind="ExternalInput").ap()
    bv = nc.dram_tensor("bv", (CH,), f16, kind="ExternalInput").ap()
    cmask = nc.dram_tensor("cmask", (C, C), f16, kind="ExternalInput").ap()
    bdmask = nc.dram_tensor("bdmask", (CH, W17), f32, kind="ExternalInput").ap()
    hmask = nc.dram_tensor("hmask", (CH, HEADS), f32, kind="ExternalInput").ap()
    yo = nc.dram_tensor("yo", (L * n_seq, CH), f16, kind="ExternalOutput").ap()

    x3 = xq.rearrange("(t n) f -> t n f", n=n_seq)
    y3 = yo.rearrange("(t n) f -> t n f", n=n_seq)

    with tile.TileContext(nc) as tc, ExitStack() as ctx:
        cpool = ctx.enter_context(tc.tile_pool(name="consts", bufs=1))
        wq_sb = cpool.tile([F, CH], f16, tag="wq")
        wk_sb = cpool.tile([F, CH], f16, tag="wk")
        wv_sb = cpool.tile([F, CH], f16, tag="wv")
        nc.sync.dma_start(wq_sb[:], wq)
        nc.sync.dma_start(wk_sb[:], wk)
        nc.sync.dma_start(wv_sb[:], wv)
        bq_sb = cpool.tile([1, CH], f16, tag="bq")
        bk_sb = cpool.tile([1, CH], f16, tag="bk")
        bv_sb = cpool.tile([1, CH], f16, tag="bv")
        nc.sync.dma_start(bq_sb[:], bq.rearrange("(a f) -> a f", a=1))
        nc.sync.dma_start(bk_sb[:], bk.rearrange("(a f) -> a f", a=1))
        nc.sync.dma_start(bv_sb[:], bv.rearrange("(a f) -> a f", a=1))
        ones_sb = cpool.tile([1, C], f16, tag="ones")
        nc.vector.memset(ones_sb[:], 1.0)
        cm_sb = cpool.tile([C, C], f16, tag="cmask")
        nc.sync.dma_start(cm_sb[:], cmask)
        bd_sb = cpool.tile([CH, W17], f32, tag="bdmask")
        nc.sync.dma_start(bd_sb[:], bdmask)
        hm_sb = cpool.tile([CH, HEADS], f32, tag="hmask")
        nc.sync.dma_start(hm_sb[:], hmask)

        xpool = ctx.enter_context(tc.tile_pool(name="x", bufs=3))
        phipool = ctx.enter_context(tc.tile_pool(name="phi", bufs=3))
        spool = ctx.enter_context(tc.tile_pool(name="sacc", bufs=1))
        tpool = ctx.enter_context(tc.tile_pool(name="tmp", bufs=2))
        opool = ctx.enter_context(tc.tile_pool(name="out", bufs=3))
        ps_proj = ctx.enter_context(tc.tile_pool(name="psproj", bufs=4, space="PSUM"))
        ps_at = ctx.enter_context(tc.tile_pool(name="psat", bufs=1, space="PSUM"))
        ps_acc = ctx.enter_context(tc.tile_pool(name="psacc", bufs=1, space="PSUM"))
        ps_inta = ctx.enter_context(tc.tile_pool(name="psinta", bufs=1, space="PSUM"))
        ps_g = ctx.enter_context(tc.tile_pool(name="psg", bufs=1, space="PSUM"))

        def phi(dst, ps):
            # phi(x) = elu(x) + 1 = relu(x) + exp(min(x, 0))
            shape = [ps.shape[0], ps.shape[1]]
            a = tpool.tile(shape, f32, tag="phia")
            b = tpool.tile(shape, f32, tag="phib")
            nc.scalar.activation(a[:], ps[:], Relu)
            nc.vector.tensor_scalar_min(b[:], ps[:], 0.0)
            nc.scalar.activation(b[:], b[:], Exp)
            nc.vector.tensor_add(dst[:], a[:], b[:])

        for n in range(n_seq):
            S_acc = spool.tile([CH, W17], f32, tag="sacc")
            nc.vector.memset(S_acc[:], 0.0)
            for c in range(NC):
                # load x chunk transposed: [F, C] (tokens of seq n, chunk c)
                xT = xpool.tile([F, C], f16, tag="xT")
                nc.sync.dma_start(xT[:], x3[c * C:(c + 1) * C, n, :], transpose=True)

                # projections (+ rank-1 bias add)
                qT_ps = ps_proj.tile([CH, C], f32, tag="proj")
                kT_ps = ps_proj.tile([CH, C], f32, tag="proj")
                kt_ps = ps_proj.tile([C, CH], f32, tag="proj")
                vt_ps = ps_proj.tile([C, CH], f32, tag="proj")
                nc.tensor.matmul(qT_ps[:], wq_sb[:], xT[:], start=True, stop=False)
                nc.tensor.matmul(qT_ps[:], bq_sb[:], ones_sb[:], start=False, stop=True)
                nc.tensor.matmul(kT_ps[:], wk_sb[:], xT[:], start=True, stop=False)
                nc.tensor.matmul(kT_ps[:], bk_sb[:], ones_sb[:], start=False, stop=True)
                nc.tensor.matmul(kt_ps[:], xT[:], wk_sb[:], start=True, stop=False)
                nc.tensor.matmul(kt_ps[:], ones_sb[:], bk_sb[:], start=False, stop=True)
                nc.tensor.matmul(vt_ps[:], xT[:], wv_sb[:], start=True, stop=False)
                nc.tensor.matmul(vt_ps[:], ones_sb[:], bv_sb[:], start=False, stop=True)

                q16 = phipool.tile([CH, C], f16, tag="q16")   # phi(q)^T [chan, tok]
                k16 = phipool.tile([CH, C], f16, tag="k16")   # phi(k)^T [chan, tok]
                kt16 = phipool.tile([C, CH], f16, tag="kt16")  # phi(k) [tok, chan]
                phi(q16, qT_ps)
                phi(k16, kT_ps)
                phi(kt16, kt_ps)

                # v_aug [tok, 136]: per head h cols h*17..h*17+15 = v_h, col h*17+16 = 1
                vaug = phipool.tile([C, W17], f16, tag="vaug")
                va = vaug[:].rearrange("p (h j) -> p h j", j=17)
                vs = vt_ps[:].rearrange("p (h j) -> p h j", j=16)
                nc.vector.tensor_copy(va[:, :, 0:16], vs)
                nc.vector.memset(va[:, :, 16:17], 1.0)

                # inter-chunk: acc[t, :] = phi(q)_t @ S_prev (block-diag interleaved)
                s16 = phipool.tile([CH, W17], f16, tag="s16")
                nc.vector.tensor_copy(s16[:], S_acc[:])
                acc_ps = ps_acc.tile([C, W17], f32, tag="acc")
                nc.tensor.matmul(acc_ps[:], q16[:], s16[:], start=True, stop=True)

                # intra-chunk per head: A^T = (k.head_mask)^T q (K=128, head-
                # masked k zeroes cross-head terms), mask causal, A_m^T.T@[v|1]
                inta_ps = ps_inta.tile([C, W17], f32, tag="inta")
                for h in range(HEADS):
                    kh = tpool.tile([CH, C], f16, tag="kh")
                    nc.vector.tensor_scalar_mul(kh[:], k16[:], hm_sb[:, h:h + 1])
                    at_ps = ps_at.tile([C, C], f32, tag="at")
                    nc.tensor.matmul(
                        at_ps[:], kh[:], q16[:], start=True, stop=True,
                    )
                    am = tpool.tile([C, C], f16, tag="am")
                    nc.vector.tensor_mul(am[:], at_ps[:], cm_sb[:])
                    nc.tensor.matmul(
                        inta_ps[:, h * 17:h * 17 + 17],
                        am[:],
                        vaug[:, h * 17:h * 17 + 17],
                        start=True, stop=True,
                    )

                # KV gram for this chunk + masked accumulate into S
                g_ps = ps_g.tile([CH, W17], f32, tag="g")
                nc.tensor.matmul(g_ps[:], kt16[:], vaug[:], start=True, stop=True)
                gt = tpool.tile([CH, W17], f32, tag="gt")
                nc.vector.tensor_mul(gt[:], g_ps[:], bd_sb[:])
                nc.vector.tensor_add(S_acc[:], S_acc[:], gt[:])

                # normalize: out = (num_inter + num_intra) / (den_i + den_x + eps)
                # DVE reads at most one PSUM operand: stage intra to SBUF first.
                inta_sb = tpool.tile([C, W17], f32, tag="intasb")
                nc.vector.tensor_copy(inta_sb[:], inta_ps[:])
                accv = acc_ps[:].rearrange("p (h j) -> p h j", j=17)
                intav = inta_sb[:].rearrange("p (h j) -> p h j", j=17)
                den = tpool.tile([C, HEADS], f32, tag="den")
                dv = den[:].rearrange("p (h j) -> p h j", j=1)
                nc.vector.scalar_tensor_tensor(
                    dv, accv[:, :, 16:17], EPS, intav[:, :, 16:17],
                    op0=AluOp.add, op1=AluOp.add,
                )
                rec = tpool.tile([C, HEADS], f32, tag="rec")
                nc.vector.reciprocal(rec[:], den[:])
                out_sb = opool.tile([C, CH], f16, tag="out")
                for h in range(HEADS):
                    nsum = tpool.tile([C, E], f32, tag="nsum")
                    nc.vector.tensor_add(
                        nsum[:],
                        acc_ps[:, h * 17:h * 17 + 16],
                        inta_sb[:, h * 17:h * 17 + 16],
                    )
                    nc.vector.tensor_scalar_mul(
                        out_sb[:, h * 16:(h + 1) * 16],
                        nsum[:],
                        rec[:, h:h + 1],
                    )
                nc.gpsimd.dma_start(y3[c * C:(c + 1) * C, n, :], out_sb[:])

    nc.compile()
    return nc


def _consts():
    cmask = np.triu(np.ones((C, C), np.float16))  # cmask[s,t] = 1 if s<=t
    bd = np.zeros((CH, W17), np.float32)
    for h in range(HEADS):
        bd[h * 16:(h + 1) * 16, h * 17:(h + 1) * 17] = 1.0
    hm = np.zeros((CH, HEADS), np.float32)
    for h in range(HEADS):
        hm[h * 16:(h + 1) * 16, h] = 1.0
    return cmask, bd, hm


_RUNNER = None


def _make_runner():
    import jax
    from jax.sharding import Mesh, NamedSharding, PartitionSpec

    try:
        from jax.experimental.shard_map import shard_map
    except ImportError:
        from jax.shard_map import shard_map

    import concourse.mybir as mybir
    from concourse.bass2jax import (
        _bass_exec_p,
        install_neuronx_cc_hook,
        partition_id_tensor,
    )

    install_neuronx_cc_hook()
    nc = build_nc(NSEQ)

    partition_name = (
        nc.partition_id_tensor.name if nc.partition_id_tensor is not None else None
    )
    in_names: list[str] = []
    out_names: list[str] = []
    out_avals = []
    zero_outs = []
    for alloc in nc.m.functions[0].allocations:
        if not isinstance(alloc, mybir.MemoryLocationSet):
            continue
        name = alloc.memorylocations[0].name
        if alloc.kind == "ExternalInput":
            if name != partition_name:
                in_names.append(name)
        elif alloc.kind == "ExternalOutput":
            out_names.append(name)
            shape = tuple(alloc.tensor_shape)
            dtype = mybir.dt.np(alloc.dtype)
            out_avals.append(jax.core.ShapedArray(shape, dtype))
            zero_outs.append(np.zeros((8 * shape[0], *shape[1:]), dtype))
    n_params = len(in_names)
    all_in_names = in_names + out_names
    if partition_name is not None:
        all_in_names = all_in_names + [partition_name]

    def _body(*args):
        operands = list(args)
        if partition_name is not None:
            operands.append(partition_id_tensor())
        outs = _bass_exec_p.bind(
            *operands,
            out_avals=tuple(out_avals),
            in_names=tuple(all_in_names),
            out_names=tuple(out_names),
            lowering_input_output_aliases=(),
            sim_require_finite=True,
            sim_require_nnan=True,
            nc=nc,
        )
        return tuple(outs)

    devices = jax.devices()[:8]
    mesh = Mesh(np.asarray(devices), ("core",))
    spec = NamedSharding(mesh, PartitionSpec("core"))
    nin = n_params + len(out_names)
    sharded = jax.jit(
        shard_map(
            _body,
            mesh=mesh,
            in_specs=(PartitionSpec("core"),) * nin,
            out_specs=(PartitionSpec("core"),) * len(out_names),
            check_rep=False,
        ),
        keep_unused=True,
    )

    cmask, bd, hm = _consts()
    persist = {
        "cmask": jax.device_put(np.tile(cmask, (8, 1)), spec),
        "bdmask": jax.device_put(np.tile(bd, (8, 1)), spec),
        "hmask": jax.device_put(np.tile(hm, (8, 1)), spec),
    }
    zeros_dev = [jax.device_put(z, spec) for z in zero_outs]

    def run(query, Wq, bq_, Wk, bk_, Wv, bv_):
        vals = {
            "xq": query.reshape(8 * L * NSEQ, F).astype(np.float16),
            "wq": np.tile(np.asarray(Wq, np.float16), (8, 1)),
            "wk": np.tile(np.asarray(Wk, np.float16), (8, 1)),
            "wv": np.tile(np.asarray(Wv, np.float16), (8, 1)),
            "bq": np.tile(np.asarray(bq_, np.float16), 8),
            "bk": np.tile(np.asarray(bk_, np.float16), 8),
            "bv": np.tile(np.asarray(bv_, np.float16), 8),
        }
        args = [persist.get(nm) if nm in persist else vals[nm] for nm in in_names]
        out = sharded(*args, *zeros_dev)
        res = np.asarray(out[0])  # (8*L*NSEQ, CH) fp16
        return res.reshape(8, L, NSEQ, CH).astype(np.float32)

    return run


def kernel(query, Wq, bq, Wk, bk, Wv, bv):
    global _RUNNER
    if _RUNNER is None:
        _RUNNER = _make_runner()
    return _RUNNER(np.asarray(query, np.float32), Wq, bq, Wk, bk, Wv, bv)
